# revision 1
# baseline (speedup 1.0000x reference)
"""Trainium2 Bass kernel for nn_MaxAggregator (GNN max message passing).

Computation (see reference):
    seg_max = segment_max(x[col], row, N); agg = where(deg>0, seg_max, x)
    out = agg @ W.T + b

Strategy (8 NeuronCores, SPMD, no collectives):
  - Destination nodes sharded contiguously: core c owns rows
    [c*12500, (c+1)*12500).
  - Neighbor features are fetched with `dma_gather` (SWDGE gather of 256B
    rows from HBM).  The x table is stored in bf16 with TWO nodes packed
    per 256B row, so the random-access footprint is 12.8MB, which stays
    inside the memory-side cache (measured: ~4-5ns per gathered row vs
    ~15-30ns for a 25.6MB f32 table).
  - int16 gather indices only reach 32767 rows, so the pair-table is split
    into 2 chunks of ~25000 pair-rows (+1 sentinel row of -1e30 for
    layering holes).  Streams are further split by source parity (which
    128B half of the gathered 256B column holds the wanted node), giving
    4 (chunk,parity) groups.
  - The node->(chunk,parity) assignment is optimized on the host (greedy
    balanced 4-coloring of sources) to minimize sum_dst max_g deg_g(dst),
    which sets the padded stream length.
  - Host-side layered layout per group: destinations sorted by max group
    degree get fixed slots (partition s%128, acc column s//128); layer k
    holds each destination's k-th group-edge (edges sorted ascending by
    pair-row for DRAM locality), holes point at the sentinel row.
  - Device: per 1024-idx dma_gather call (round-robin over 4 SWDGE queues
    = 4 Q7 descriptor-gen core pairs), DVE tensor_tensor max folds the
    relevant 128B half into the bf16 accumulator.
  - Output: per 2 blocks, one PE transpose (bf16), ACT copy PSUM->SBUF,
    PE matmul against replicated W^T, DVE bias-add, DMA out (f32).
  - Host unpermutes slot order -> node ids and concatenates cores.
"""

import os
import sys

import numpy as np

_RL_REPO = "/opt/trn_rl_repo"
if _RL_REPO not in sys.path and os.path.isdir(_RL_REPO):
    sys.path.insert(0, _RL_REPO)

import ml_dtypes

import concourse.bacc as bacc
import concourse.mybir as mybir
import concourse.tile as tile
from concourse.bass_utils import run_bass_kernel_spmd
from concourse.masks import make_identity

F32 = mybir.dt.float32
BF16 = mybir.dt.bfloat16
I16 = mybir.dt.int16

N_NODES = 100000
D = 64
N_CORES = 8
N_CHUNKS = 2
NG = 4                 # (chunk, parity) stream groups
CALL_COLS = 8          # 1024 idxs per dma_gather call (single-packet max)
NQ = 4                 # SWDGE queues (Q7 core pairs)
NEG = -1.0e30


def _round128(n):
    return -(-n // 128) * 128


def balance_groups(row, col, n_nodes=N_NODES, n_iter=8, seed=0):
    """Greedy vectorized search for a balanced node->group map minimizing
    sum_dst max_g deg_g (cuts layering holes)."""
    rng = np.random.default_rng(seed)
    deg = np.bincount(row, minlength=n_nodes)
    zero = np.nonzero(deg == 0)[0]
    rows_all = np.concatenate([row, zero])
    cols_all = np.concatenate([col, zero])
    E = len(rows_all)
    cap = n_nodes // NG

    g_node = rng.permutation(np.arange(n_nodes) % NG).astype(np.int64)
    deg_g = np.zeros((n_nodes, NG), dtype=np.int64)
    np.add.at(deg_g, (rows_all, g_node[cols_all]), 1)

    src_order = np.argsort(cols_all, kind="stable")
    src_sorted = cols_all[src_order]
    dst_sorted = rows_all[src_order]
    best_obj = int(deg_g.max(axis=1).sum())
    best_g = g_node.copy()
    for _ in range(n_iter):
        dvec = deg_g[dst_sorted]
        ge = g_node[src_sorted]
        cur_max = dvec.max(axis=1)
        gain = np.zeros((n_nodes, NG), dtype=np.int64)
        ar = np.arange(E)
        for g2 in range(NG):
            nv = dvec.copy()
            np.subtract.at(nv, (ar, ge), 1)
            nv[ar, g2] += 1
            np.add.at(gain[:, g2], src_sorted, nv.max(axis=1) - cur_max)
        bg = gain.argmin(axis=1)
        bgain = gain[np.arange(n_nodes), bg]
        movers = np.nonzero((bg != g_node) & (bgain < 0))[0]
        if len(movers) == 0:
            break
        movers = movers[np.argsort(bgain[movers])]
        sizes = np.bincount(g_node, minlength=NG)
        take = []
        for n in movers[:40000]:
            gt_ = bg[n]
            if sizes[gt_] >= cap + 300:
                continue
            sizes[g_node[n]] -= 1
            sizes[gt_] += 1
            take.append(n)
        take = np.asarray(take, dtype=np.int64)
        g_node[take] = bg[take]
        deg_g = np.zeros((n_nodes, NG), dtype=np.int64)
        np.add.at(deg_g, (rows_all, g_node[cols_all]), 1)
        o = int(deg_g.max(axis=1).sum())
        if o < best_obj:
            best_obj = o
            best_g = g_node.copy()
    return best_g


def table_layout(g_node, n_nodes=N_NODES):
    """Node -> (pair-row, parity, chunk) from the group map."""
    chunk_of = g_node >> 1
    par_of = g_node & 1
    prow_of = np.zeros(n_nodes, dtype=np.int64)
    for g in range(NG):
        ns = np.nonzero(g_node == g)[0]
        prow_of[ns] = np.arange(len(ns))
    return prow_of, par_of, chunk_of


def make_plan(row, col, g_node, n_nodes=N_NODES, n_cores=N_CORES,
              tiebreak=False, split=True, nsplit=32):
    """Host-side planning.  Returns (plan, idx_sb [ncores,128,T16] i16,
    perm [ncores, nloc])."""
    nloc = n_nodes // n_cores
    deg = np.bincount(row, minlength=n_nodes)
    zero = np.nonzero(deg == 0)[0].astype(row.dtype)
    rows_all = np.concatenate([row, zero])
    cols_all = np.concatenate([col, zero])

    prow_of, par_of, chunk_of = table_layout(g_node)
    maxgrp = int(np.bincount(g_node, minlength=NG).max())
    hole = maxgrp                 # any index >= group size hits a NEG row
    aug = maxgrp + 1

    core_e = rows_all // nloc
    prow_e = prow_of[cols_all]
    grp_e = g_node[cols_all]

    deg_g = np.zeros((n_nodes, NG), dtype=np.int64)
    np.add.at(deg_g, (rows_all, grp_e), 1)
    key = deg_g.max(axis=1)

    if tiebreak:
        # secondary sort by dst's smallest source pair-row: bands the layer
        # streams for DRAM locality within equal-key runs
        first_prow = np.full(n_nodes, 1 << 30, dtype=np.int64)
        np.minimum.at(first_prow, rows_all, prow_e)
    pos_of = np.empty(n_nodes, dtype=np.int64)
    perm = np.empty((n_cores, nloc), dtype=np.int64)
    for c in range(n_cores):
        sl = slice(c * nloc, (c + 1) * nloc)
        dloc = key[sl]
        if tiebreak:
            order = np.lexsort((first_prow[sl], -dloc))
        else:
            order = np.argsort(-dloc, kind="stable")
        perm[c] = order + c * nloc
        pos_of[c * nloc + order] = np.arange(nloc)
    s_e = pos_of[rows_all]

    # rank k within (dst, grp), edges sorted ascending by pair-row
    gid = rows_all.astype(np.int64) * NG + grp_e
    sort_i = np.lexsort((prow_e, gid))
    gs = gid[sort_i]
    first = np.r_[True, gs[1:] != gs[:-1]]
    start = np.maximum.accumulate(np.where(first, np.arange(len(gs)), 0))
    k_e = np.empty(len(gs), dtype=np.int64)
    k_e[sort_i] = np.arange(len(gs)) - start

    # per (core, slot, grp) degree
    degq = np.zeros((n_cores, nloc, NG), dtype=np.int64)
    np.add.at(degq, (core_e, s_e, grp_e), 1)

    kmax_g = degq.reshape(-1, NG).max(axis=0)
    off = {}
    L_gk = {}
    T = 0
    for g in range(NG):
        sufmax = np.maximum.accumulate(degq[:, ::-1, g], axis=1)[:, ::-1]
        for k in range(int(kmax_g[g])):
            n_k = (sufmax > k).sum(axis=1)        # per-core prefix length
            L = _round128(int(n_k.max()))
            off[(g, k)] = T
            L_gk[(g, k)] = L
            T += L
    assert T % 128 == 0

    idx_flat = np.full((n_cores, T), hole, dtype=np.int16)
    off_arr = np.zeros((NG, int(kmax_g.max())), dtype=np.int64)
    for (g, k), o in off.items():
        off_arr[g, k] = o
    dest_pos = off_arr[grp_e, k_e] + s_e
    idx_flat[core_e, dest_pos] = prow_e.astype(np.int16)

    T16 = T // 16
    idx_sb = np.ascontiguousarray(
        np.tile(idx_flat.reshape(n_cores, T16, 16).transpose(0, 2, 1),
                (1, 8, 1)))

    # calls: per group, chop its stream into pieces of <= CALL_COLS*128 idxs
    calls = []   # (g, off_blk(128s), ncols, [(acc_c0, tile_c0, nb)])
    for g in range(NG):
        g_lo = off[(g, 0)]
        g_hi = off[(g, int(kmax_g[g]) - 1)] + L_gk[(g, int(kmax_g[g]) - 1)]
        lo = g_lo
        while lo < g_hi:
            hi = min(lo + CALL_COLS * 128, g_hi)
            segs = []
            for k in range(int(kmax_g[g])):
                a, b = off[(g, k)], off[(g, k)] + L_gk[(g, k)]
                s0, s1 = max(lo, a), min(hi, b)
                if s0 < s1:
                    segs.append(((s0 - a) // 128, (s0 - lo) // 128,
                                 (s1 - s0) // 128))
            calls.append((g, lo // 128, (hi - lo) // 128, segs))
            lo = hi

    plan = dict(T=T, T16=T16, calls=calls, nloc=nloc, aug=aug)
    if split:
        # regroup the SAME calls into nsplit phases by acc block range; a
        # call goes to the EARLIEST phase whose block range it touches, so
        # each phase's output stage runs only after every max-contribution
        # to its blocks has landed.
        nblk = -(-nloc // 128)
        bounds = [round(i * nblk / nsplit) for i in range(nsplit + 1)]
        phases = [[] for _ in range(nsplit)]
        for call in calls:
            segs = call[3]
            blk_min = min(a0 for (a0, t0, nb) in segs)
            for p in range(nsplit):
                if blk_min < bounds[p + 1]:
                    phases[p].append(call)
                    break
        plan["phases"] = [(phases[p], bounds[p], bounds[p + 1])
                          for p in range(nsplit)]
    return plan, idx_sb, perm


def build_kernel_body(tc, out_ap, xaug_ap, idx_ap, wtb_ap, bias_ap, plan,
                      repeat=1, d=D, gbufs=4):
    nc = tc.nc
    aug = plan["aug"]
    nloc = plan["nloc"]
    nblk = -(-nloc // 128)
    T16 = plan["T16"]

    from contextlib import ExitStack
    es = ExitStack()
    const = es.enter_context(tc.tile_pool(name="const", bufs=1))
    gpool = es.enter_context(tc.tile_pool(name="gather", bufs=gbufs))
    xtp = es.enter_context(tc.tile_pool(name="xt", bufs=3))
    otp = es.enter_context(tc.tile_pool(name="ot", bufs=2))
    ppool = es.enter_context(tc.tile_pool(name="psum", bufs=2, space="PSUM"))
    ppool2 = es.enter_context(tc.tile_pool(name="psum2", bufs=4, space="PSUM"))

    idx_sb = const.tile([128, T16], I16)
    nc.sync.dma_start(idx_sb[:], idx_ap)
    wtb_sb = const.tile([d + 1, d], BF16)
    nc.sync.dma_start(wtb_sb[:], wtb_ap)
    ident = const.tile([128, 128], BF16)
    make_identity(nc, ident[:])
    acc = const.tile([128, nblk * d], BF16)
    bias_bc = const.tile([128, d], F32)
    nc.sync.dma_start(bias_bc[:], bias_ap)
    wtb_dup = const.tile([128, d], BF16)
    nc.vector.tensor_copy(out=wtb_dup[:d, :], in_=wtb_sb[:d, :])
    nc.vector.tensor_copy(out=wtb_dup[d:2 * d, :], in_=wtb_sb[:d, :])
    ones2 = const.tile([128, 128], BF16)
    nc.vector.memset(ones2[:], 1.0)
    bias2 = const.tile([128, d], BF16)
    nc.vector.tensor_copy(out=bias2[:1, :], in_=wtb_sb[d:d + 1, :])
    nc.vector.tensor_copy(out=bias2[d:d + 1, :], in_=wtb_sb[d:d + 1, :])

    def emit_calls(call_list, ci0):
        ci = ci0
        for (g, off_blk, ncols, segs) in call_list:
            gt = gpool.tile([128, CALL_COLS, 2 * d], BF16, tag="gt")
            n_idx = ncols * 128
            q = g >> 1
            par = g & 1
            nc.gpsimd.dma_gather(
                gt[:, :ncols, :],
                xaug_ap[q * aug:(q + 1) * aug, :],
                idx_sb[:, off_blk * 8:off_blk * 8 + n_idx // 16],
                n_idx,
                n_idx,
                2 * d,
                queue_num=ci % NQ,
            )
            ci += 1
            for (acc_c0, tile_c0, nb) in segs:
                nc.vector.tensor_tensor(
                    out=acc[:, acc_c0 * d:(acc_c0 + nb) * d],
                    in0=acc[:, acc_c0 * d:(acc_c0 + nb) * d],
                    in1=gt[:, tile_c0:tile_c0 + nb, par * d:(par + 1) * d],
                    op=mybir.AluOpType.max,
                )
        return ci

    def emit_out_dvefree(blk_lo, blk_hi):
        # PE transpose + ACT copies + bias preloaded into PSUM; no DVE ops
        GRP = 8
        for g0 in range(blk_lo, blk_hi, GRP):
            gn = min(GRP, blk_hi - g0)
            ot = otp.tile([128, GRP, d], F32, tag="ot")
            for j0 in range(0, gn, 2):
                blk = g0 + j0
                nb2 = min(2, gn - j0)
                pt = ppool.tile([128, 128], BF16, tag="pt")
                nc.tensor.transpose(
                    pt[:nb2 * d, :],
                    acc[:, blk * d:(blk + nb2) * d], ident[:])
                st = xtp.tile([128, 128], BF16, tag="st")
                nc.scalar.copy(out=st[:nb2 * d, :], in_=pt[:nb2 * d, :])
                for h in range(nb2):
                    po = ppool2.tile([128, d], F32, tag="po")
                    nc.tensor.matmul(po[:], ones2[h * d:h * d + 1, :],
                                     bias2[h * d:h * d + 1, :],
                                     start=True, stop=False)
                    nc.tensor.matmul(po[:], st[h * d:(h + 1) * d, :],
                                     wtb_dup[h * d:(h + 1) * d, :],
                                     start=False, stop=True)
                    nc.scalar.copy(out=ot[:, j0 + h, :], in_=po[:])
            hbm = out_ap[g0 * 128:(g0 + gn) * 128, :].rearrange(
                "(b p) f -> p b f", p=128)
            nc.sync.dma_start(hbm, ot[:, :gn, :])

    for _rep in range(repeat):
        nc.vector.memset(acc[:], NEG)
        if "phases" in plan:
            ci = 0
            for call_list, blk_lo, blk_hi in plan["phases"]:
                ci = emit_calls(call_list, ci)
                emit_out_dvefree(blk_lo, blk_hi)
            continue
        emit_calls(plan["calls"], 0)

        GRP = 8
        for g0 in range(0, nblk, GRP):
            gn = min(GRP, nblk - g0)
            ot = otp.tile([128, GRP, d], F32, tag="ot")
            for j0 in range(0, gn, 2):
                blk = g0 + j0
                nb2 = min(2, gn - j0)
                pt = ppool.tile([128, 128], BF16, tag="pt")
                nc.tensor.transpose(
                    pt[:, :nb2 * d],
                    acc[:, blk * d:(blk + nb2) * d], ident[:])
                st = xtp.tile([128, 128], BF16, tag="st")
                nc.scalar.copy(out=st[:, :nb2 * d], in_=pt[:, :nb2 * d])
                for h in range(nb2):
                    po = ppool2.tile([128, d], F32, tag="po")
                    nc.tensor.matmul(po[:], st[h * d:(h + 1) * d, :],
                                     wtb_dup[h * d:(h + 1) * d, :],
                                     start=True, stop=True)
                    nc.vector.tensor_tensor(
                        out=ot[:, j0 + h, :], in0=po[:], in1=bias_bc[:],
                        op=mybir.AluOpType.add)
            hbm = out_ap[g0 * 128:(g0 + gn) * 128, :].rearrange(
                "(b p) f -> p b f", p=128)
            nc.sync.dma_start(hbm, ot[:, :gn, :])
    es.close()


def build_nc(plan, repeat=1, d=D, scratch=49152, gbufs=6):
    nc = bacc.Bacc("TRN2", target_bir_lowering=False, debug=False,
                   num_swdge_queues=NQ, dynamic_dma_scratch_size=scratch)
    nloc = plan["nloc"]
    nblk = -(-nloc // 128)
    xaug = nc.dram_tensor("xaug", [N_CHUNKS * plan["aug"], 2 * d], BF16,
                          kind="ExternalInput")
    idxs = nc.dram_tensor("idxs", [128, plan["T16"]], I16,
                          kind="ExternalInput")
    wtb = nc.dram_tensor("wtb", [d + 1, d], BF16, kind="ExternalInput")
    bias = nc.dram_tensor("bias", [128, d], F32, kind="ExternalInput")
    out = nc.dram_tensor("out", [nblk * 128, d], F32, kind="ExternalOutput")
    with tile.TileContext(nc) as tc:
        build_kernel_body(tc, out.ap(), xaug.ap(), idxs.ap(), wtb.ap(),
                          bias.ap(), plan, repeat=repeat, d=d, gbufs=gbufs)
    nc.compile()
    return nc


def make_xaug(x, g_node):
    x16 = x.astype(ml_dtypes.bfloat16)
    prow_of, par_of, chunk_of = table_layout(g_node)
    aug = int(np.bincount(g_node, minlength=NG).max()) + 1
    xp = np.full((N_CHUNKS, aug, 2, D), NEG, dtype=ml_dtypes.bfloat16)
    xp[chunk_of, prow_of, par_of] = x16
    return xp.reshape(N_CHUNKS * aug, 2 * D)


def make_wtb(W, b):
    return np.concatenate([W.T, b[None, :]]).astype(ml_dtypes.bfloat16)


def make_in_maps(x, W, b, g_node, idx_sb):
    xaug = make_xaug(x, g_node)
    wtb = make_wtb(W, b)
    bias_bc = np.tile(b.astype(np.float32)[None, :], (128, 1))
    return [
        {"xaug": xaug, "idxs": np.ascontiguousarray(idx_sb[c]), "wtb": wtb,
         "bias": bias_bc}
        for c in range(N_CORES)
    ]


_trace = bool(int(os.environ.get("GNN_TRACE", "0")))
_last_results = None


def kernel(x, W, b, edge_index):
    global _last_results
    x = np.asarray(x, dtype=np.float32)
    W = np.asarray(W, dtype=np.float32)
    b = np.asarray(b, dtype=np.float32)
    edge_index = np.asarray(edge_index)
    row = edge_index[0].astype(np.int64)
    col = edge_index[1].astype(np.int64)

    g_node = balance_groups(row, col)
    plan, idx_sb, perm = make_plan(row, col, g_node)
    nc = build_nc(plan)

    in_maps = make_in_maps(x, W, b, g_node, idx_sb)
    res = run_bass_kernel_spmd(nc, in_maps, core_ids=list(range(N_CORES)),
                               trace=_trace)
    _last_results = res
    nloc = plan["nloc"]
    out = np.empty((N_NODES, D), dtype=np.float32)
    for c in range(N_CORES):
        out[perm[c]] = res.results[c]["out"][:nloc]
    return out



# revision 2
# speedup vs baseline: 6.5346x; 6.5346x over previous
"""Trainium2 Bass kernel for nn_MaxAggregator (GNN max message passing).

Computation (see reference):
    seg_max = segment_max(x[col], row, N); agg = where(deg>0, seg_max, x)
    out = agg @ W.T + b

Strategy (8 NeuronCores, SPMD, no collectives):
  - Destination nodes sharded contiguously: core c owns rows
    [c*12500, (c+1)*12500).
  - Neighbor features are fetched with `dma_gather` (SWDGE gather of 256B
    rows from HBM).  The x table is stored in bf16 with TWO nodes packed
    per 256B row, so the random-access footprint is 12.8MB, which stays
    inside the memory-side cache (measured: ~4-5ns per gathered row vs
    ~15-30ns for a 25.6MB f32 table).
  - int16 gather indices only reach 32767 rows, so the pair-table is split
    into 2 chunks of ~25000 pair-rows (+1 sentinel row of -1e30 for
    layering holes).  Streams are further split by source parity (which
    128B half of the gathered 256B column holds the wanted node), giving
    4 (chunk,parity) groups.
  - The node->(chunk,parity) assignment is optimized on the host (greedy
    balanced 4-coloring of sources) to minimize sum_dst max_g deg_g(dst),
    which sets the padded stream length.
  - Host-side layered layout per group: destinations sorted by max group
    degree get fixed slots (partition s%128, acc column s//128); layer k
    holds each destination's k-th group-edge (edges sorted ascending by
    pair-row for DRAM locality), holes point at the sentinel row.
  - Device: per 1024-idx dma_gather call (round-robin over 4 SWDGE queues
    = 4 Q7 descriptor-gen core pairs), DVE tensor_tensor max folds the
    relevant 128B half into the bf16 accumulator.
  - Output: per 2 blocks, one PE transpose (bf16), ACT copy PSUM->SBUF,
    PE matmul against replicated W^T, DVE bias-add, DMA out (f32).
  - Host unpermutes slot order -> node ids and concatenates cores.
"""

import os
import sys

import numpy as np

_RL_REPO = "/opt/trn_rl_repo"
if _RL_REPO not in sys.path and os.path.isdir(_RL_REPO):
    sys.path.insert(0, _RL_REPO)

import ml_dtypes

import concourse.bacc as bacc
import concourse.mybir as mybir
import concourse.tile as tile
from concourse.bass_utils import run_bass_kernel_spmd
from concourse.masks import make_identity

F32 = mybir.dt.float32
BF16 = mybir.dt.bfloat16
I16 = mybir.dt.int16

N_NODES = 100000
D = 64
N_CORES = 8
N_CHUNKS = 2
NG = 4                 # (chunk, parity) stream groups
CALL_COLS = 8          # 1024 idxs per dma_gather call (single-packet max)
NQ = 4                 # SWDGE queues (Q7 core pairs)
NEG = -1.0e30


def _round128(n):
    return -(-n // 128) * 128


def balance_groups(row, col, n_nodes=N_NODES, n_iter=20, seed=0):
    """Greedy vectorized search for a balanced node->group map minimizing
    sum_dst max_g deg_g (cuts layering holes)."""
    rng = np.random.default_rng(seed)
    deg = np.bincount(row, minlength=n_nodes)
    zero = np.nonzero(deg == 0)[0]
    rows_all = np.concatenate([row, zero])
    cols_all = np.concatenate([col, zero])
    E = len(rows_all)
    cap = n_nodes // NG

    g_node = rng.permutation(np.arange(n_nodes) % NG).astype(np.int64)
    deg_g = np.zeros((n_nodes, NG), dtype=np.int64)
    np.add.at(deg_g, (rows_all, g_node[cols_all]), 1)

    src_order = np.argsort(cols_all, kind="stable")
    src_sorted = cols_all[src_order]
    dst_sorted = rows_all[src_order]
    best_obj = int(deg_g.max(axis=1).sum())
    best_g = g_node.copy()
    for _ in range(n_iter):
        dvec = deg_g[dst_sorted]
        ge = g_node[src_sorted]
        cur_max = dvec.max(axis=1)
        gain = np.zeros((n_nodes, NG), dtype=np.int64)
        ar = np.arange(E)
        for g2 in range(NG):
            nv = dvec.copy()
            np.subtract.at(nv, (ar, ge), 1)
            nv[ar, g2] += 1
            np.add.at(gain[:, g2], src_sorted, nv.max(axis=1) - cur_max)
        bg = gain.argmin(axis=1)
        bgain = gain[np.arange(n_nodes), bg]
        movers = np.nonzero((bg != g_node) & (bgain < 0))[0]
        if len(movers) == 0:
            break
        movers = movers[np.argsort(bgain[movers])]
        sizes = np.bincount(g_node, minlength=NG)
        take = []
        for n in movers[:100000]:
            gt_ = bg[n]
            if sizes[gt_] >= cap + 600:
                continue
            sizes[g_node[n]] -= 1
            sizes[gt_] += 1
            take.append(n)
        take = np.asarray(take, dtype=np.int64)
        g_node[take] = bg[take]
        deg_g = np.zeros((n_nodes, NG), dtype=np.int64)
        np.add.at(deg_g, (rows_all, g_node[cols_all]), 1)
        o = int(deg_g.max(axis=1).sum())
        if o < best_obj:
            best_obj = o
            best_g = g_node.copy()
    return best_g


def table_layout(g_node, n_nodes=N_NODES):
    """Node -> (pair-row, parity, chunk) from the group map."""
    chunk_of = g_node >> 1
    par_of = g_node & 1
    prow_of = np.zeros(n_nodes, dtype=np.int64)
    for g in range(NG):
        ns = np.nonzero(g_node == g)[0]
        prow_of[ns] = np.arange(len(ns))
    return prow_of, par_of, chunk_of


def make_plan(row, col, g_node, n_nodes=N_NODES, n_cores=N_CORES,
              tiebreak=False, split=True, nsplit=32):
    """Host-side planning.  Returns (plan, idx_sb [ncores,128,T16] i16,
    perm [ncores, nloc])."""
    nloc = n_nodes // n_cores
    deg = np.bincount(row, minlength=n_nodes)
    zero = np.nonzero(deg == 0)[0].astype(row.dtype)
    rows_all = np.concatenate([row, zero])
    cols_all = np.concatenate([col, zero])

    prow_of, par_of, chunk_of = table_layout(g_node)
    maxgrp = int(np.bincount(g_node, minlength=NG).max())
    hole = maxgrp                 # any index >= group size hits a NEG row
    aug = maxgrp + 1

    core_e = rows_all // nloc
    prow_e = prow_of[cols_all]
    grp_e = g_node[cols_all]

    deg_g = np.zeros((n_nodes, NG), dtype=np.int64)
    np.add.at(deg_g, (rows_all, grp_e), 1)
    key = deg_g.max(axis=1)

    if tiebreak:
        # secondary sort by dst's smallest source pair-row: bands the layer
        # streams for DRAM locality within equal-key runs
        first_prow = np.full(n_nodes, 1 << 30, dtype=np.int64)
        np.minimum.at(first_prow, rows_all, prow_e)
    pos_of = np.empty(n_nodes, dtype=np.int64)
    perm = np.empty((n_cores, nloc), dtype=np.int64)
    for c in range(n_cores):
        sl = slice(c * nloc, (c + 1) * nloc)
        dloc = key[sl]
        if tiebreak:
            order = np.lexsort((first_prow[sl], -dloc))
        else:
            order = np.argsort(-dloc, kind="stable")
        perm[c] = order + c * nloc
        pos_of[c * nloc + order] = np.arange(nloc)
    s_e = pos_of[rows_all]

    # rank k within (dst, grp), edges sorted ascending by pair-row
    gid = rows_all.astype(np.int64) * NG + grp_e
    sort_i = np.lexsort((prow_e, gid))
    gs = gid[sort_i]
    first = np.r_[True, gs[1:] != gs[:-1]]
    start = np.maximum.accumulate(np.where(first, np.arange(len(gs)), 0))
    k_e = np.empty(len(gs), dtype=np.int64)
    k_e[sort_i] = np.arange(len(gs)) - start

    # per (core, slot, grp) degree
    degq = np.zeros((n_cores, nloc, NG), dtype=np.int64)
    np.add.at(degq, (core_e, s_e, grp_e), 1)

    kmax_g = degq.reshape(-1, NG).max(axis=0)
    off = {}
    L_gk = {}
    T = 0
    for g in range(NG):
        sufmax = np.maximum.accumulate(degq[:, ::-1, g], axis=1)[:, ::-1]
        for k in range(int(kmax_g[g])):
            n_k = (sufmax > k).sum(axis=1)        # per-core prefix length
            L = _round128(int(n_k.max()))
            off[(g, k)] = T
            L_gk[(g, k)] = L
            T += L
    assert T % 128 == 0

    idx_flat = np.full((n_cores, T), hole, dtype=np.int16)
    off_arr = np.zeros((NG, int(kmax_g.max())), dtype=np.int64)
    for (g, k), o in off.items():
        off_arr[g, k] = o
    dest_pos = off_arr[grp_e, k_e] + s_e
    idx_flat[core_e, dest_pos] = prow_e.astype(np.int16)

    T16 = T // 16
    idx_sb = np.ascontiguousarray(
        np.tile(idx_flat.reshape(n_cores, T16, 16).transpose(0, 2, 1),
                (1, 8, 1)))

    # calls: per group, chop its stream into pieces of <= CALL_COLS*128 idxs
    calls = []   # (g, off_blk(128s), ncols, [(acc_c0, tile_c0, nb)])
    for g in range(NG):
        g_lo = off[(g, 0)]
        g_hi = off[(g, int(kmax_g[g]) - 1)] + L_gk[(g, int(kmax_g[g]) - 1)]
        lo = g_lo
        while lo < g_hi:
            hi = min(lo + CALL_COLS * 128, g_hi)
            segs = []
            for k in range(int(kmax_g[g])):
                a, b = off[(g, k)], off[(g, k)] + L_gk[(g, k)]
                s0, s1 = max(lo, a), min(hi, b)
                if s0 < s1:
                    segs.append(((s0 - a) // 128, (s0 - lo) // 128,
                                 (s1 - s0) // 128))
            calls.append((g, lo // 128, (hi - lo) // 128, segs))
            lo = hi

    plan = dict(T=T, T16=T16, calls=calls, nloc=nloc, aug=aug)
    if split:
        # regroup the SAME calls into nsplit phases by acc block range; a
        # call goes to the EARLIEST phase whose block range it touches, so
        # each phase's output stage runs only after every max-contribution
        # to its blocks has landed.
        nblk = -(-nloc // 128)
        bounds = [round(i * nblk / nsplit) for i in range(nsplit + 1)]
        phases = [[] for _ in range(nsplit)]
        for call in calls:
            segs = call[3]
            blk_min = min(a0 for (a0, t0, nb) in segs)
            for p in range(nsplit):
                if blk_min < bounds[p + 1]:
                    phases[p].append(call)
                    break
        plan["phases"] = [(phases[p], bounds[p], bounds[p + 1])
                          for p in range(nsplit)]
    return plan, idx_sb, perm


def build_kernel_body(tc, out_ap, xaug_ap, idx_ap, wtb_ap, bias_ap, plan,
                      repeat=1, d=D, gbufs=4):
    nc = tc.nc
    aug = plan["aug"]
    nloc = plan["nloc"]
    nblk = -(-nloc // 128)
    T16 = plan["T16"]

    from contextlib import ExitStack
    es = ExitStack()
    const = es.enter_context(tc.tile_pool(name="const", bufs=1))
    gpool = es.enter_context(tc.tile_pool(name="gather", bufs=gbufs))
    xtp = es.enter_context(tc.tile_pool(name="xt", bufs=3))
    otp = es.enter_context(tc.tile_pool(name="ot", bufs=2))
    ppool = es.enter_context(tc.tile_pool(name="psum", bufs=2, space="PSUM"))
    ppool2 = es.enter_context(tc.tile_pool(name="psum2", bufs=4, space="PSUM"))

    idx_sb = const.tile([128, T16], I16)
    nc.sync.dma_start(idx_sb[:], idx_ap)
    wtb_sb = const.tile([d + 1, d], BF16)
    nc.sync.dma_start(wtb_sb[:], wtb_ap)
    ident = const.tile([128, 128], BF16)
    make_identity(nc, ident[:])
    acc = const.tile([128, nblk * d], BF16)
    bias_bc = const.tile([128, d], F32)
    nc.sync.dma_start(bias_bc[:], bias_ap)
    wtb_dup = const.tile([128, d], BF16)
    nc.vector.tensor_copy(out=wtb_dup[:d, :], in_=wtb_sb[:d, :])
    nc.vector.tensor_copy(out=wtb_dup[d:2 * d, :], in_=wtb_sb[:d, :])
    ones2 = const.tile([128, 128], BF16)
    nc.vector.memset(ones2[:], 1.0)
    bias2 = const.tile([128, d], BF16)
    nc.vector.tensor_copy(out=bias2[:1, :], in_=wtb_sb[d:d + 1, :])
    nc.vector.tensor_copy(out=bias2[d:d + 1, :], in_=wtb_sb[d:d + 1, :])

    def emit_calls(call_list, ci0):
        ci = ci0
        for (g, off_blk, ncols, segs) in call_list:
            gt = gpool.tile([128, CALL_COLS, 2 * d], BF16, tag="gt")
            n_idx = ncols * 128
            q = g >> 1
            par = g & 1
            nc.gpsimd.dma_gather(
                gt[:, :ncols, :],
                xaug_ap[q * aug:(q + 1) * aug, :],
                idx_sb[:, off_blk * 8:off_blk * 8 + n_idx // 16],
                n_idx,
                n_idx,
                2 * d,
                queue_num=ci % NQ,
            )
            ci += 1
            for (acc_c0, tile_c0, nb) in segs:
                nc.vector.tensor_tensor(
                    out=acc[:, acc_c0 * d:(acc_c0 + nb) * d],
                    in0=acc[:, acc_c0 * d:(acc_c0 + nb) * d],
                    in1=gt[:, tile_c0:tile_c0 + nb, par * d:(par + 1) * d],
                    op=mybir.AluOpType.max,
                )
        return ci

    def emit_out_dvefree(blk_lo, blk_hi):
        # PE transpose + ACT copies + bias preloaded into PSUM; no DVE ops
        GRP = 8
        for g0 in range(blk_lo, blk_hi, GRP):
            gn = min(GRP, blk_hi - g0)
            ot = otp.tile([128, GRP, d], F32, tag="ot")
            for j0 in range(0, gn, 2):
                blk = g0 + j0
                nb2 = min(2, gn - j0)
                pt = ppool.tile([128, 128], BF16, tag="pt")
                nc.tensor.transpose(
                    pt[:nb2 * d, :],
                    acc[:, blk * d:(blk + nb2) * d], ident[:])
                st = xtp.tile([128, 128], BF16, tag="st")
                nc.scalar.copy(out=st[:nb2 * d, :], in_=pt[:nb2 * d, :])
                for h in range(nb2):
                    po = ppool2.tile([128, d], F32, tag="po")
                    nc.tensor.matmul(po[:], ones2[h * d:h * d + 1, :],
                                     bias2[h * d:h * d + 1, :],
                                     start=True, stop=False)
                    nc.tensor.matmul(po[:], st[h * d:(h + 1) * d, :],
                                     wtb_dup[h * d:(h + 1) * d, :],
                                     start=False, stop=True)
                    nc.scalar.copy(out=ot[:, j0 + h, :], in_=po[:])
            hbm = out_ap[g0 * 128:(g0 + gn) * 128, :].rearrange(
                "(b p) f -> p b f", p=128)
            nc.sync.dma_start(hbm, ot[:, :gn, :])

    for _rep in range(repeat):
        nc.vector.memset(acc[:], NEG)
        if "phases" in plan:
            ci = 0
            for call_list, blk_lo, blk_hi in plan["phases"]:
                ci = emit_calls(call_list, ci)
                emit_out_dvefree(blk_lo, blk_hi)
            continue
        emit_calls(plan["calls"], 0)

        GRP = 8
        for g0 in range(0, nblk, GRP):
            gn = min(GRP, nblk - g0)
            ot = otp.tile([128, GRP, d], F32, tag="ot")
            for j0 in range(0, gn, 2):
                blk = g0 + j0
                nb2 = min(2, gn - j0)
                pt = ppool.tile([128, 128], BF16, tag="pt")
                nc.tensor.transpose(
                    pt[:, :nb2 * d],
                    acc[:, blk * d:(blk + nb2) * d], ident[:])
                st = xtp.tile([128, 128], BF16, tag="st")
                nc.scalar.copy(out=st[:, :nb2 * d], in_=pt[:, :nb2 * d])
                for h in range(nb2):
                    po = ppool2.tile([128, d], F32, tag="po")
                    nc.tensor.matmul(po[:], st[h * d:(h + 1) * d, :],
                                     wtb_dup[h * d:(h + 1) * d, :],
                                     start=True, stop=True)
                    nc.vector.tensor_tensor(
                        out=ot[:, j0 + h, :], in0=po[:], in1=bias_bc[:],
                        op=mybir.AluOpType.add)
            hbm = out_ap[g0 * 128:(g0 + gn) * 128, :].rearrange(
                "(b p) f -> p b f", p=128)
            nc.sync.dma_start(hbm, ot[:, :gn, :])
    es.close()


def build_nc(plan, repeat=1, d=D, scratch=49152, gbufs=6):
    nc = bacc.Bacc("TRN2", target_bir_lowering=False, debug=False,
                   num_swdge_queues=NQ, dynamic_dma_scratch_size=scratch)
    nloc = plan["nloc"]
    nblk = -(-nloc // 128)
    xaug = nc.dram_tensor("xaug", [N_CHUNKS * plan["aug"], 2 * d], BF16,
                          kind="ExternalInput")
    idxs = nc.dram_tensor("idxs", [128, plan["T16"]], I16,
                          kind="ExternalInput")
    wtb = nc.dram_tensor("wtb", [d + 1, d], BF16, kind="ExternalInput")
    bias = nc.dram_tensor("bias", [128, d], F32, kind="ExternalInput")
    out = nc.dram_tensor("out", [nblk * 128, d], F32, kind="ExternalOutput")
    with tile.TileContext(nc) as tc:
        build_kernel_body(tc, out.ap(), xaug.ap(), idxs.ap(), wtb.ap(),
                          bias.ap(), plan, repeat=repeat, d=d, gbufs=gbufs)
    nc.compile()
    return nc


def make_xaug(x, g_node):
    x16 = x.astype(ml_dtypes.bfloat16)
    prow_of, par_of, chunk_of = table_layout(g_node)
    aug = int(np.bincount(g_node, minlength=NG).max()) + 1
    xp = np.full((N_CHUNKS, aug, 2, D), NEG, dtype=ml_dtypes.bfloat16)
    xp[chunk_of, prow_of, par_of] = x16
    return xp.reshape(N_CHUNKS * aug, 2 * D)


def make_wtb(W, b):
    return np.concatenate([W.T, b[None, :]]).astype(ml_dtypes.bfloat16)


def make_in_maps(x, W, b, g_node, idx_sb):
    xaug = make_xaug(x, g_node)
    wtb = make_wtb(W, b)
    bias_bc = np.tile(b.astype(np.float32)[None, :], (128, 1))
    return [
        {"xaug": xaug, "idxs": np.ascontiguousarray(idx_sb[c]), "wtb": wtb,
         "bias": bias_bc}
        for c in range(N_CORES)
    ]


_trace = bool(int(os.environ.get("GNN_TRACE", "0")))
_last_results = None


def kernel(x, W, b, edge_index):
    global _last_results
    x = np.asarray(x, dtype=np.float32)
    W = np.asarray(W, dtype=np.float32)
    b = np.asarray(b, dtype=np.float32)
    edge_index = np.asarray(edge_index)
    row = edge_index[0].astype(np.int64)
    col = edge_index[1].astype(np.int64)

    g_node = balance_groups(row, col)
    plan, idx_sb, perm = make_plan(row, col, g_node)
    nc = build_nc(plan)

    in_maps = make_in_maps(x, W, b, g_node, idx_sb)
    res = run_bass_kernel_spmd(nc, in_maps, core_ids=list(range(N_CORES)),
                               trace=_trace)
    _last_results = res
    nloc = plan["nloc"]
    out = np.empty((N_NODES, D), dtype=np.float32)
    for c in range(N_CORES):
        out[perm[c]] = res.results[c]["out"][:nloc]
    return out



# revision 3
# speedup vs baseline: 8.5853x; 1.3138x over previous
"""Trainium2 Bass kernel for nn_MaxAggregator (GNN max message passing).

Computation (see reference):
    seg_max = segment_max(x[col], row, N); agg = where(deg>0, seg_max, x)
    out = agg @ W.T + b

Strategy (8 NeuronCores, SPMD, no collectives):
  - Destination nodes sharded contiguously: core c owns rows
    [c*12500, (c+1)*12500).
  - Neighbor features are fetched with `dma_gather` (SWDGE gather of 256B
    rows from HBM).  The x table is stored in bf16 with TWO nodes packed
    per 256B row, so the random-access footprint is 12.8MB, which stays
    inside the memory-side cache (measured: ~4-5ns per gathered row vs
    ~15-30ns for a 25.6MB f32 table).
  - int16 gather indices only reach 32767 rows, so the pair-table is split
    into 2 chunks of ~25000 pair-rows (+1 sentinel row of -1e30 for
    layering holes).  Streams are further split by source parity (which
    128B half of the gathered 256B column holds the wanted node), giving
    4 (chunk,parity) groups.
  - The node->(chunk,parity) assignment is optimized on the host (greedy
    balanced 4-coloring of sources) to minimize sum_dst max_g deg_g(dst),
    which sets the padded stream length.
  - Host-side layered layout per group: destinations sorted by max group
    degree get fixed slots (partition s%128, acc column s//128); layer k
    holds each destination's k-th group-edge (edges sorted ascending by
    pair-row for DRAM locality), holes point at the sentinel row.
  - Device: per 1024-idx dma_gather call (round-robin over 4 SWDGE queues
    = 4 Q7 descriptor-gen core pairs), DVE tensor_tensor max folds the
    relevant 128B half into the bf16 accumulator.
  - Output: per 2 blocks, one PE transpose (bf16), ACT copy PSUM->SBUF,
    PE matmul against replicated W^T, DVE bias-add, DMA out (f32).
  - Host unpermutes slot order -> node ids and concatenates cores.
"""

import os
import sys

import numpy as np

_RL_REPO = "/opt/trn_rl_repo"
if _RL_REPO not in sys.path and os.path.isdir(_RL_REPO):
    sys.path.insert(0, _RL_REPO)

import ml_dtypes

import concourse.bacc as bacc
import concourse.mybir as mybir
import concourse.tile as tile
from concourse.bass_utils import run_bass_kernel_spmd
from concourse.masks import make_identity

F32 = mybir.dt.float32
BF16 = mybir.dt.bfloat16
I16 = mybir.dt.int16

N_NODES = 100000
D = 64
N_CORES = 8
N_CHUNKS = 2
NG = 4                 # (chunk, parity) stream groups
CALL_COLS = 8          # 1024 idxs per dma_gather call (single-packet max)
NQ = 4                 # SWDGE queues (Q7 core pairs)
NEG = -1.0e30


def _round128(n):
    return -(-n // 128) * 128


def balance_groups(row, col, n_nodes=N_NODES, n_iter=20, n_shake=24,
                   seed=0):
    """Greedy vectorized search for a balanced node->group map minimizing
    sum_dst max_g deg_g (cuts layering holes).  After the plain descent
    plateaus, a shake phase (every 4th iteration also applies a random
    half of the zero-gain moves) escapes it; measured obj 567828->509324
    (T 290560->~258k rows/core) on the reference graph."""
    rng = np.random.default_rng(seed)
    deg = np.bincount(row, minlength=n_nodes)
    zero = np.nonzero(deg == 0)[0]
    rows_all = np.concatenate([row, zero])
    cols_all = np.concatenate([col, zero])
    E = len(rows_all)
    cap = n_nodes // NG

    g_node = rng.permutation(np.arange(n_nodes) % NG).astype(np.int64)
    deg_g = np.zeros((n_nodes, NG), dtype=np.int64)
    np.add.at(deg_g, (rows_all, g_node[cols_all]), 1)

    src_order = np.argsort(cols_all, kind="stable")
    src_sorted = cols_all[src_order]
    dst_sorted = rows_all[src_order]
    best_obj = int(deg_g.max(axis=1).sum())
    best_g = g_node.copy()
    ar = np.arange(E)

    def descend(iters, shake_every):
        nonlocal g_node, deg_g, best_obj, best_g
        for it in range(iters):
            dvec = deg_g[dst_sorted]
            ge = g_node[src_sorted]
            cur_max = dvec.max(axis=1)
            gain = np.zeros((n_nodes, NG), dtype=np.int64)
            for g2 in range(NG):
                nv = dvec.copy()
                np.subtract.at(nv, (ar, ge), 1)
                nv[ar, g2] += 1
                np.add.at(gain[:, g2], src_sorted, nv.max(axis=1) - cur_max)
            bg = gain.argmin(axis=1)
            bgain = gain[np.arange(n_nodes), bg]
            shake = shake_every and ((it + 1) % shake_every == 0)
            movers = np.nonzero((bg != g_node)
                                & (bgain < (1 if shake else 0)))[0]
            if len(movers) == 0:
                break
            if shake:
                movers = movers[rng.random(len(movers)) < 0.5]
            movers = movers[np.argsort(bgain[movers])]
            sizes = np.bincount(g_node, minlength=NG)
            take = []
            for n in movers[:100000]:
                gt_ = bg[n]
                if sizes[gt_] >= cap + 600:
                    continue
                sizes[g_node[n]] -= 1
                sizes[gt_] += 1
                take.append(n)
            take = np.asarray(take, dtype=np.int64)
            g_node[take] = bg[take]
            deg_g = np.zeros((n_nodes, NG), dtype=np.int64)
            np.add.at(deg_g, (rows_all, g_node[cols_all]), 1)
            o = int(deg_g.max(axis=1).sum())
            if o < best_obj:
                best_obj = o
                best_g = g_node.copy()

    descend(n_iter, 0)
    descend(n_shake, 4)
    return best_g


def table_layout(g_node, n_nodes=N_NODES):
    """Node -> (pair-row, parity, chunk) from the group map."""
    chunk_of = g_node >> 1
    par_of = g_node & 1
    prow_of = np.zeros(n_nodes, dtype=np.int64)
    for g in range(NG):
        ns = np.nonzero(g_node == g)[0]
        prow_of[ns] = np.arange(len(ns))
    return prow_of, par_of, chunk_of


def make_plan(row, col, g_node, n_nodes=N_NODES, n_cores=N_CORES,
              tiebreak=False, split=True, nsplit=32):
    """Host-side planning.  Returns (plan, idx_sb [ncores,128,T16] i16,
    perm [ncores, nloc])."""
    nloc = n_nodes // n_cores
    deg = np.bincount(row, minlength=n_nodes)
    zero = np.nonzero(deg == 0)[0].astype(row.dtype)
    rows_all = np.concatenate([row, zero])
    cols_all = np.concatenate([col, zero])

    prow_of, par_of, chunk_of = table_layout(g_node)
    maxgrp = int(np.bincount(g_node, minlength=NG).max())
    hole = maxgrp                 # any index >= group size hits a NEG row
    aug = maxgrp + 1

    core_e = rows_all // nloc
    prow_e = prow_of[cols_all]
    grp_e = g_node[cols_all]

    deg_g = np.zeros((n_nodes, NG), dtype=np.int64)
    np.add.at(deg_g, (rows_all, grp_e), 1)
    key = deg_g.max(axis=1)

    if tiebreak:
        # secondary sort by dst's smallest source pair-row: bands the layer
        # streams for DRAM locality within equal-key runs
        first_prow = np.full(n_nodes, 1 << 30, dtype=np.int64)
        np.minimum.at(first_prow, rows_all, prow_e)
    pos_of = np.empty(n_nodes, dtype=np.int64)
    perm = np.empty((n_cores, nloc), dtype=np.int64)
    for c in range(n_cores):
        sl = slice(c * nloc, (c + 1) * nloc)
        dloc = key[sl]
        if tiebreak:
            order = np.lexsort((first_prow[sl], -dloc))
        else:
            order = np.argsort(-dloc, kind="stable")
        perm[c] = order + c * nloc
        pos_of[c * nloc + order] = np.arange(nloc)
    s_e = pos_of[rows_all]

    # rank k within (dst, grp), edges sorted ascending by pair-row
    gid = rows_all.astype(np.int64) * NG + grp_e
    sort_i = np.lexsort((prow_e, gid))
    gs = gid[sort_i]
    first = np.r_[True, gs[1:] != gs[:-1]]
    start = np.maximum.accumulate(np.where(first, np.arange(len(gs)), 0))
    k_e = np.empty(len(gs), dtype=np.int64)
    k_e[sort_i] = np.arange(len(gs)) - start

    # per (core, slot, grp) degree
    degq = np.zeros((n_cores, nloc, NG), dtype=np.int64)
    np.add.at(degq, (core_e, s_e, grp_e), 1)

    kmax_g = degq.reshape(-1, NG).max(axis=0)
    off = {}
    L_gk = {}
    T = 0
    for g in range(NG):
        sufmax = np.maximum.accumulate(degq[:, ::-1, g], axis=1)[:, ::-1]
        for k in range(int(kmax_g[g])):
            n_k = (sufmax > k).sum(axis=1)        # per-core prefix length
            L = _round128(int(n_k.max()))
            off[(g, k)] = T
            L_gk[(g, k)] = L
            T += L
    assert T % 128 == 0

    idx_flat = np.full((n_cores, T), hole, dtype=np.int16)
    off_arr = np.zeros((NG, int(kmax_g.max())), dtype=np.int64)
    for (g, k), o in off.items():
        off_arr[g, k] = o
    dest_pos = off_arr[grp_e, k_e] + s_e
    idx_flat[core_e, dest_pos] = prow_e.astype(np.int16)

    T16 = T // 16
    idx_sb = np.ascontiguousarray(
        np.tile(idx_flat.reshape(n_cores, T16, 16).transpose(0, 2, 1),
                (1, 8, 1)))

    # calls: per group, chop its stream into pieces of <= CALL_COLS*128 idxs
    calls = []   # (g, off_blk(128s), ncols, [(acc_c0, tile_c0, nb)])
    for g in range(NG):
        g_lo = off[(g, 0)]
        g_hi = off[(g, int(kmax_g[g]) - 1)] + L_gk[(g, int(kmax_g[g]) - 1)]
        lo = g_lo
        while lo < g_hi:
            hi = min(lo + CALL_COLS * 128, g_hi)
            segs = []
            for k in range(int(kmax_g[g])):
                a, b = off[(g, k)], off[(g, k)] + L_gk[(g, k)]
                s0, s1 = max(lo, a), min(hi, b)
                if s0 < s1:
                    segs.append(((s0 - a) // 128, (s0 - lo) // 128,
                                 (s1 - s0) // 128))
            calls.append((g, lo // 128, (hi - lo) // 128, segs))
            lo = hi

    plan = dict(T=T, T16=T16, calls=calls, nloc=nloc, aug=aug)
    if split:
        # regroup the SAME calls into nsplit phases by acc block range; a
        # call goes to the EARLIEST phase whose block range it touches, so
        # each phase's output stage runs only after every max-contribution
        # to its blocks has landed.
        nblk = -(-nloc // 128)
        bounds = [round(i * nblk / nsplit) for i in range(nsplit + 1)]
        phases = [[] for _ in range(nsplit)]
        for call in calls:
            segs = call[3]
            blk_min = min(a0 for (a0, t0, nb) in segs)
            for p in range(nsplit):
                if blk_min < bounds[p + 1]:
                    phases[p].append(call)
                    break
        plan["phases"] = [(phases[p], bounds[p], bounds[p + 1])
                          for p in range(nsplit)]
    return plan, idx_sb, perm


def build_kernel_body(tc, out_ap, xaug_ap, idx_ap, wtb_ap, bias_ap, plan,
                      repeat=1, d=D, gbufs=4):
    nc = tc.nc
    aug = plan["aug"]
    nloc = plan["nloc"]
    nblk = -(-nloc // 128)
    T16 = plan["T16"]

    from contextlib import ExitStack
    es = ExitStack()
    const = es.enter_context(tc.tile_pool(name="const", bufs=1))
    gpool = es.enter_context(tc.tile_pool(name="gather", bufs=gbufs))
    xtp = es.enter_context(tc.tile_pool(name="xt", bufs=3))
    otp = es.enter_context(tc.tile_pool(name="ot", bufs=2))
    ppool = es.enter_context(tc.tile_pool(name="psum", bufs=2, space="PSUM"))
    ppool2 = es.enter_context(tc.tile_pool(name="psum2", bufs=4, space="PSUM"))

    idx_sb = const.tile([128, T16], I16)
    nc.sync.dma_start(idx_sb[:], idx_ap)
    wtb_sb = const.tile([d + 1, d], BF16)
    nc.sync.dma_start(wtb_sb[:], wtb_ap)
    ident = const.tile([128, 128], BF16)
    make_identity(nc, ident[:])
    acc = const.tile([128, nblk * d], BF16)
    bias_bc = const.tile([128, d], F32)
    nc.sync.dma_start(bias_bc[:], bias_ap)
    wtb_dup = const.tile([128, d], BF16)
    nc.vector.tensor_copy(out=wtb_dup[:d, :], in_=wtb_sb[:d, :])
    nc.vector.tensor_copy(out=wtb_dup[d:2 * d, :], in_=wtb_sb[:d, :])
    ones2 = const.tile([128, 128], BF16)
    nc.vector.memset(ones2[:], 1.0)
    bias2 = const.tile([128, d], BF16)
    nc.vector.tensor_copy(out=bias2[:1, :], in_=wtb_sb[d:d + 1, :])
    nc.vector.tensor_copy(out=bias2[d:d + 1, :], in_=wtb_sb[d:d + 1, :])

    def emit_calls(call_list, ci0):
        ci = ci0
        for (g, off_blk, ncols, segs) in call_list:
            gt = gpool.tile([128, CALL_COLS, 2 * d], BF16, tag="gt")
            n_idx = ncols * 128
            q = g >> 1
            par = g & 1
            nc.gpsimd.dma_gather(
                gt[:, :ncols, :],
                xaug_ap[q * aug:(q + 1) * aug, :],
                idx_sb[:, off_blk * 8:off_blk * 8 + n_idx // 16],
                n_idx,
                n_idx,
                2 * d,
                queue_num=ci % NQ,
            )
            ci += 1
            for (acc_c0, tile_c0, nb) in segs:
                nc.vector.tensor_tensor(
                    out=acc[:, acc_c0 * d:(acc_c0 + nb) * d],
                    in0=acc[:, acc_c0 * d:(acc_c0 + nb) * d],
                    in1=gt[:, tile_c0:tile_c0 + nb, par * d:(par + 1) * d],
                    op=mybir.AluOpType.max,
                )
        return ci

    def emit_out_dvefree(blk_lo, blk_hi):
        # PE transpose + ACT copies + bias preloaded into PSUM; no DVE ops
        GRP = 8
        for g0 in range(blk_lo, blk_hi, GRP):
            gn = min(GRP, blk_hi - g0)
            ot = otp.tile([128, GRP, d], F32, tag="ot")
            for j0 in range(0, gn, 2):
                blk = g0 + j0
                nb2 = min(2, gn - j0)
                pt = ppool.tile([128, 128], BF16, tag="pt")
                nc.tensor.transpose(
                    pt[:nb2 * d, :],
                    acc[:, blk * d:(blk + nb2) * d], ident[:])
                st = xtp.tile([128, 128], BF16, tag="st")
                nc.scalar.copy(out=st[:nb2 * d, :], in_=pt[:nb2 * d, :])
                for h in range(nb2):
                    po = ppool2.tile([128, d], F32, tag="po")
                    nc.tensor.matmul(po[:], ones2[h * d:h * d + 1, :],
                                     bias2[h * d:h * d + 1, :],
                                     start=True, stop=False)
                    nc.tensor.matmul(po[:], st[h * d:(h + 1) * d, :],
                                     wtb_dup[h * d:(h + 1) * d, :],
                                     start=False, stop=True)
                    nc.scalar.copy(out=ot[:, j0 + h, :], in_=po[:])
            hbm = out_ap[g0 * 128:(g0 + gn) * 128, :].rearrange(
                "(b p) f -> p b f", p=128)
            nc.sync.dma_start(hbm, ot[:, :gn, :])

    for _rep in range(repeat):
        nc.vector.memset(acc[:], NEG)
        if "phases" in plan:
            ci = 0
            for call_list, blk_lo, blk_hi in plan["phases"]:
                ci = emit_calls(call_list, ci)
                emit_out_dvefree(blk_lo, blk_hi)
            continue
        emit_calls(plan["calls"], 0)

        GRP = 8
        for g0 in range(0, nblk, GRP):
            gn = min(GRP, nblk - g0)
            ot = otp.tile([128, GRP, d], F32, tag="ot")
            for j0 in range(0, gn, 2):
                blk = g0 + j0
                nb2 = min(2, gn - j0)
                pt = ppool.tile([128, 128], BF16, tag="pt")
                nc.tensor.transpose(
                    pt[:, :nb2 * d],
                    acc[:, blk * d:(blk + nb2) * d], ident[:])
                st = xtp.tile([128, 128], BF16, tag="st")
                nc.scalar.copy(out=st[:, :nb2 * d], in_=pt[:, :nb2 * d])
                for h in range(nb2):
                    po = ppool2.tile([128, d], F32, tag="po")
                    nc.tensor.matmul(po[:], st[h * d:(h + 1) * d, :],
                                     wtb_dup[h * d:(h + 1) * d, :],
                                     start=True, stop=True)
                    nc.vector.tensor_tensor(
                        out=ot[:, j0 + h, :], in0=po[:], in1=bias_bc[:],
                        op=mybir.AluOpType.add)
            hbm = out_ap[g0 * 128:(g0 + gn) * 128, :].rearrange(
                "(b p) f -> p b f", p=128)
            nc.sync.dma_start(hbm, ot[:, :gn, :])
    es.close()


def build_nc(plan, repeat=1, d=D, scratch=49152, gbufs=6):
    nc = bacc.Bacc("TRN2", target_bir_lowering=False, debug=False,
                   num_swdge_queues=NQ, dynamic_dma_scratch_size=scratch)
    nloc = plan["nloc"]
    nblk = -(-nloc // 128)
    xaug = nc.dram_tensor("xaug", [N_CHUNKS * plan["aug"], 2 * d], BF16,
                          kind="ExternalInput")
    idxs = nc.dram_tensor("idxs", [128, plan["T16"]], I16,
                          kind="ExternalInput")
    wtb = nc.dram_tensor("wtb", [d + 1, d], BF16, kind="ExternalInput")
    bias = nc.dram_tensor("bias", [128, d], F32, kind="ExternalInput")
    out = nc.dram_tensor("out", [nblk * 128, d], F32, kind="ExternalOutput")
    with tile.TileContext(nc) as tc:
        build_kernel_body(tc, out.ap(), xaug.ap(), idxs.ap(), wtb.ap(),
                          bias.ap(), plan, repeat=repeat, d=d, gbufs=gbufs)
    nc.compile()
    return nc


def make_xaug(x, g_node):
    x16 = x.astype(ml_dtypes.bfloat16)
    prow_of, par_of, chunk_of = table_layout(g_node)
    aug = int(np.bincount(g_node, minlength=NG).max()) + 1
    xp = np.full((N_CHUNKS, aug, 2, D), NEG, dtype=ml_dtypes.bfloat16)
    xp[chunk_of, prow_of, par_of] = x16
    return xp.reshape(N_CHUNKS * aug, 2 * D)


def make_wtb(W, b):
    return np.concatenate([W.T, b[None, :]]).astype(ml_dtypes.bfloat16)


def make_in_maps(x, W, b, g_node, idx_sb):
    xaug = make_xaug(x, g_node)
    wtb = make_wtb(W, b)
    bias_bc = np.tile(b.astype(np.float32)[None, :], (128, 1))
    return [
        {"xaug": xaug, "idxs": np.ascontiguousarray(idx_sb[c]), "wtb": wtb,
         "bias": bias_bc}
        for c in range(N_CORES)
    ]


_trace = bool(int(os.environ.get("GNN_TRACE", "0")))
_last_results = None


def kernel(x, W, b, edge_index):
    global _last_results
    x = np.asarray(x, dtype=np.float32)
    W = np.asarray(W, dtype=np.float32)
    b = np.asarray(b, dtype=np.float32)
    edge_index = np.asarray(edge_index)
    row = edge_index[0].astype(np.int64)
    col = edge_index[1].astype(np.int64)

    g_node = balance_groups(row, col)
    plan, idx_sb, perm = make_plan(row, col, g_node)
    nc = build_nc(plan)

    in_maps = make_in_maps(x, W, b, g_node, idx_sb)
    res = run_bass_kernel_spmd(nc, in_maps, core_ids=list(range(N_CORES)),
                               trace=_trace)
    _last_results = res
    nloc = plan["nloc"]
    out = np.empty((N_NODES, D), dtype=np.float32)
    for c in range(N_CORES):
        out[perm[c]] = res.results[c]["out"][:nloc]
    return out

